# revision 5
# baseline (speedup 1.0000x reference)
"""Delay-and-sum beamformer on 8 TRN2 NeuronCores.

Problem: x[16, 100000, 128] f32 -> out[b, t] = mean_s x[b, t + d_s, s],
d_s = round(s * sin(30deg) / 2) in [0, 32] (zero-padded past t = T-1).

Sharding: pure data parallel over batch (2 batches per core).

Per-core layout ("stripe" scheme): for each batch, partition p owns time
rows [784*p, 784*(p+1)) of a zero-padded T_pad = 100352 signal, loaded in
~112-row chunks through a 3-slot ring as [row, sensor] (row pitch 128).
Loads are SWDGE DMAs that cast f32 -> bf16 in flight: HBM traffic stays
exactly the (padded) input + output, SBUF write traffic halves, and the
DVE reads 16-bit data (2x element rate).

The delay structure groups sensors as {0,1,2} (d=0), {4k-1..4k+2} (d=k,
k=1..31), {127} (d=32).  Compute is two passes, sized so DVE time per
chunk (~12us) hides under the chunk load (~18us):

  1. G-pass (per chunk, while resident): 33 group sums per time row,
     written DIAG-ALIGNED into a persistent bf16 buffer H[t, d] =
     G[t+d, d] (so pass 2 reads contiguously): one strided DVE reduce
     with a negative-stride output for k=1..31, one small DVE reduce for
     d=0, one ACT copy for sensor 127.  H covers rows [-32, 784) per
     partition; the skewed writes fill the [-32, 0) pre-rows naturally.
  2. diag-pass: out[tau] = sum_d H[tau][d] -- one fully contiguous
     33-run DVE reduce (bf16 in, f32 out) -- then ACT in-place 1/128.

The cross-partition halo (partition p's rows [752, 784) overlap p+1's
rows [0, 32)) lives in H-space: one tiny partition-shifted SBUF->SBUF
DMA (268 KB/batch) copies p+1's pre-rows into p's tail after chunk 0's
G-pass; chunk 6's G-pass then overwrites the locally-valid triangle of
that rectangle.  Chunk order per batch is 0, 6, 5, .., 2, then chunk 1
in two 56-row halves (shorter end-of-kernel drain).  Partition 127's
tail is memset to zero once (beyond-batch rows are zero-padded).

Stores and the shift issue on the ACT HWDGE ring; loads go through the
SWDGE (gpsimd) queue -- neither blocks the other at the sequencer.
"""

import numpy as np

B, T, S = 16, 100000, 128
NCORES = 8
BC = B // NCORES          # batches per core
LS = 784                  # stripe rows per partition (128*784 = 100352 >= T)
TP = 128 * LS             # padded rows per batch
HB = 32                   # H pre-rows (= max delay)
LC = 112                  # ring slot rows
G_W = 33                  # group-sum entries per row
SCALE = 1.0 / S

# per-batch load/compute pieces (row ranges), in processing order:
# chunk 0 first (feeds the partition-shift), then descending, the last
# chunk split in half to shorten the end-of-kernel drain.
PIECES = [(0, 112)] + [(a, a + 112) for a in range(672, 223, -112)] + [
    (168, 224),
    (112, 168),
]

_cache = {}


def _build():
    import concourse.bass as bass
    import concourse.tile as tile
    from concourse import bacc, mybir

    f32 = mybir.dt.float32
    bf16 = mybir.dt.bfloat16
    nc = bacc.Bacc("TRN2", target_bir_lowering=False, debug=False, num_devices=1)
    x = nc.dram_tensor("x", [BC * TP * S], f32, kind="ExternalInput")
    y = nc.dram_tensor("y", [BC * TP], f32, kind="ExternalOutput")

    def dram_ap(base_elem, rows):
        # [128 partitions (stripe-major), rows*S contiguous elems each]
        return bass.AP(x.ap().tensor, base_elem, [[LS * S, 128], [1, rows * S]])

    def sub_ap(t, off, dims):
        # custom AP into a tile: keep its partition dim, replace free dims
        return bass.AP(t.tensor, t.offset + off, [list(t.ap[0])] + dims)

    X = mybir.AxisListType.X

    with tile.TileContext(nc) as tc:
        from contextlib import ExitStack

        with ExitStack() as ctx:
            ring_pool = ctx.enter_context(tc.tile_pool(name="ring", bufs=1))
            h_pool = ctx.enter_context(tc.tile_pool(name="h", bufs=1))
            o_pool = ctx.enter_context(tc.tile_pool(name="o", bufs=2))

            ring = ring_pool.tile([128, 3 * LC * S], bf16)
            H = h_pool.tile([128, (LS + HB) * G_W], bf16)

            # zero all tails once; each batch's shift overwrites partitions
            # 0..126, so partition 127 (reads beyond the batch) stays zero
            nc.vector.memset(H[:, (LS - 32 + HB) * G_W : (LS + HB) * G_W], 0.0)

            load_i = 0

            def load(b, t0, t1):
                nonlocal load_i
                slot = load_i % 3
                load_i += 1
                nc.gpsimd.dma_start(
                    sub_ap(ring, slot * LC * S, [[1, (t1 - t0) * S]]),
                    dram_ap(b * TP * S + t0 * S, t1 - t0),
                )
                return slot

            def g_pass(t0, t1, slot):
                n = t1 - t0
                base = slot * LC * S
                with nc.allow_low_precision(reason="bf16 group sums, gate 2e-2"):
                    # groups k=1..31 (sensors 4k-1..4k+2), skew-written:
                    # H[r-k][k] = G_k[r]
                    nc.vector.reduce_sum(
                        sub_ap(H, (t0 + HB) * G_W - 32, [[G_W, n], [1 - G_W, 31]]),
                        sub_ap(ring, base + 3, [[S, n], [4, 31], [1, 4]]),
                        axis=X,
                    )
                    # d=0 (sensors 0..2): H[r][0]
                    nc.vector.reduce_sum(
                        sub_ap(H, (t0 + HB) * G_W, [[G_W, n]]),
                        sub_ap(ring, base, [[S, n], [1, 3]]),
                        axis=X,
                    )
                # sensor 127 (d=32): H[r-32][32], on ACT
                nc.scalar.copy(
                    sub_ap(H, (t0 + HB - 32) * G_W + 32, [[G_W, n]]),
                    sub_ap(ring, base + 127, [[S, n]]),
                )

            def diag(t0, t1, out_sb):
                n = t1 - t0
                o = out_sb[:, t0:t1]
                nc.vector.reduce_sum(
                    o,
                    sub_ap(H, (t0 + HB) * G_W, [[G_W, n], [1, G_W]]),
                    axis=X,
                )
                nc.scalar.mul(o, o, SCALE)

            for b in range(BC):
                out_sb = o_pool.tile([128, LS], f32, tag="out_sb")

                # chunk 0 first: its pre-rows feed the partition shift
                t0, t1 = PIECES[0]
                slot = load(b, t0, t1)
                g_pass(t0, t1, slot)
                nc.scalar.dma_start(
                    H[0:127, (LS - 32 + HB) * G_W : (LS + HB) * G_W],
                    H[1:128, 0 : HB * G_W],
                )

                for t0, t1 in PIECES[1:]:
                    slot = load(b, t0, t1)
                    g_pass(t0, t1, slot)
                    diag(t0, t1, out_sb)
                    if t0 == 336:
                        # top half of the batch output is complete
                        nc.scalar.dma_start(
                            bass.AP(
                                y.ap().tensor, b * TP + 336, [[LS, 128], [1, LS - 336]]
                            ),
                            out_sb[:, 336:LS],
                        )
                diag(0, 112, out_sb)
                nc.scalar.dma_start(
                    bass.AP(y.ap().tensor, b * TP, [[LS, 128], [1, 336]]),
                    out_sb[:, 0:336],
                )

    nc.compile()
    return nc


def _get_nc():
    if "nc" not in _cache:
        _cache["nc"] = _build()
    return _cache["nc"]


def kernel(microphone_array: np.ndarray) -> np.ndarray:
    from concourse.bass_utils import run_bass_kernel_spmd

    x = np.asarray(microphone_array, dtype=np.float32)
    assert x.shape == (B, T, S)
    nc = _get_nc()

    in_maps = []
    for c in range(NCORES):
        shard = np.zeros((BC, TP, S), dtype=np.float32)
        shard[:, :T] = x[c * BC : (c + 1) * BC]
        in_maps.append({"x": shard.reshape(-1)})

    res = _cache["res"] = run_bass_kernel_spmd(
        nc, in_maps, core_ids=list(range(NCORES)), trace=_cache.get("trace", False)
    )

    out = np.empty((B, T), dtype=np.float32)
    for c in range(NCORES):
        out[c * BC : (c + 1) * BC] = res.results[c]["y"].reshape(BC, TP)[:, :T]
    return out


# revision 11
# speedup vs baseline: 1.1017x; 1.1017x over previous
"""Delay-and-sum beamformer on 8 TRN2 NeuronCores.

Problem: x[16, 100000, 128] f32 -> out[b, t] = mean_s x[b, t + d_s, s],
d_s = round(s * sin(30deg) / 2) in [0, 32] (zero-padded past t = T-1).

Sharding: pure data parallel over batch (2 batches per core).

Per-core layout ("stripe" scheme): for each batch, partition p owns time
rows [784*p, 784*(p+1)) of a zero-padded T_pad = 100352 signal, loaded in
14 chunks of 56 rows through a 3-slot ring as [row, sensor] (row pitch
128 f32).  HBM traffic is the (padded) input + output only.

The delay structure groups sensors as {0,1,2} (d=0), {4k-1..4k+2} (d=k,
k=1..31), {127} (d=32).  All f32.  Work is split across three engines so
each stays well under the ~9.2us chunk load time (DVE reduce cost fits
dur ~ 124ns + 1.19ns/run + 0.77ns/elem, so fewer elems + longer runs):

  1. fold (GPSIMD): P[r, i] = x[r, 3+i] + x[r+16, 67+i], i<60 -- pairs
     group k with group k+16 at their exact 16-row delay offset, halving
     what DVE must reduce.  Two contiguous tensor_adds per chunk (the
     last 16 rows read the next chunk up, resident since chunks are
     processed descending).
  2. G-pass (DVE + ACT): per row, 18 partial sums written DIAG-ALIGNED
     into a persistent H[t][d] buffer (negative-stride outputs):
     H[t][0] = G0[t] (sensors 0-2), H[t][k] = F_k[t+k] = 4-sum of P
     (k=1..15), H[t][16] = G16[t+16] (sensors 63-66), H[t][17] =
     x127[t+32] (ACT copy).  Then out[tau] = sum_d H[tau][d].
  3. diag (DVE): one fully contiguous 18-run reduce + ACT 1/128 scale.

Cross-partition halos (partition p's window reaches 32 rows into p+1):
  - raw: the last chunk's fold reads rows [784, 800) from a 16-row
    partition-shifted copy of chunk 0 (1 MB/batch SBUF DMA, "stage").
  - H: partition p's H tail rows [752, 784) come from p+1's H pre-rows
    [-32, 0) (skewed writes fill them naturally) via one partition-
    shifted SBUF DMA after chunk 0's G-pass; the last chunk's G-pass
    then overwrites the locally-valid triangle of that rectangle.
Partition 127 reads zeros (memset once; beyond-batch rows are padding).

Chunk order per batch: chunk 0 first (minus its last-16-rows fold,
deferred to batch end when chunk 1 is resident), then 13, 12, .., 1.
Loads issue on the sync HWDGE ring; stores and shifts on the ACT ring.
"""

import numpy as np

B, T, S = 16, 100000, 128
NCORES = 8
BC = B // NCORES          # batches per core
LS = 784                  # stripe rows per partition (128*784 = 100352 >= T)
TP = 128 * LS             # padded rows per batch
HB = 32                   # H pre-rows (= max delay)
LC = 56                   # chunk rows
NCH = LS // LC            # 14 chunks per batch
W = 18                    # H entries per row
SCALE = 1.0 / S

_cache = {}


def _build():
    import concourse.bass as bass
    import concourse.tile as tile
    from concourse import bacc, mybir

    f32 = mybir.dt.float32
    nc = bacc.Bacc("TRN2", target_bir_lowering=False, debug=False, num_devices=1)
    x = nc.dram_tensor("x", [BC * TP * S], f32, kind="ExternalInput")
    y = nc.dram_tensor("y", [BC * TP], f32, kind="ExternalOutput")

    def dram_ap(base_elem, rows):
        # [128 partitions (stripe-major), rows*S contiguous elems each]
        return bass.AP(x.ap().tensor, base_elem, [[LS * S, 128], [1, rows * S]])

    def sub_ap(t, off, dims):
        # custom AP into a tile: keep its partition dim, replace free dims
        return bass.AP(t.tensor, t.offset + off, [list(t.ap[0])] + dims)

    X = mybir.AxisListType.X

    with tile.TileContext(nc) as tc:
        from contextlib import ExitStack

        with ExitStack() as ctx:
            ring_pool = ctx.enter_context(tc.tile_pool(name="ring", bufs=1))
            h_pool = ctx.enter_context(tc.tile_pool(name="h", bufs=1))
            p_pool = ctx.enter_context(tc.tile_pool(name="pp", bufs=2))
            o_pool = ctx.enter_context(tc.tile_pool(name="o", bufs=2))

            ring = ring_pool.tile([128, 3 * LC * S], f32)
            H = h_pool.tile([128, (LS + HB) * W], f32)
            stage = h_pool.tile([128, 16 * S], f32)   # next partition's rows 0..16
            PB = h_pool.tile([128, 16 * 60], f32)     # chunk 0's deferred fold tail
            c0sav = h_pool.tile([128, 16 * 60], f32)  # chunk 0 rows [40,56) cols 3:63

            # zero all H tails once; each batch's H-shift overwrites partitions
            # 0..126, so partition 127 (reads beyond the batch) stays zero.
            # Same for stage (partition 127 reads rows past the whole batch).
            nc.vector.memset(H[:, LS * W : (LS + HB) * W], 0.0)
            nc.vector.memset(stage[:], 0.0)

            load_i = 0

            def load(b, t0, t1):
                nonlocal load_i
                slot = load_i % 3
                load_i += 1
                nc.sync.dma_start(
                    sub_ap(ring, slot * LC * S, [[1, (t1 - t0) * S]]),
                    dram_ap(b * TP * S + t0 * S, t1 - t0),
                )
                return slot

            def fold1(slot, P, n):
                # P[r, i] = x[r, 3+i] + x[r+16, 67+i] for r in [0, n-16)
                base = slot * LC * S
                nc.gpsimd.tensor_add(
                    sub_ap(P, 0, [[60, n - 16], [1, 60]]),
                    sub_ap(ring, base + 3, [[S, n - 16], [1, 60]]),
                    sub_ap(ring, base + 16 * S + 67, [[S, n - 16], [1, 60]]),
                )

            def fold2(slot, rowoff, P, poff, hi_ap):
                # rows [rowoff, rowoff+16): the +16 rows live in hi_ap
                base = slot * LC * S
                nc.gpsimd.tensor_add(
                    sub_ap(P, poff * 60, [[60, 16], [1, 60]]),
                    sub_ap(ring, base + rowoff * S + 3, [[S, 16], [1, 60]]),
                    hi_ap,
                )

            def g_pass(t0, t1, slot, P):
                n = t1 - t0
                base = slot * LC * S
                # F_k (k=1..15): 4-sums of P, skew-written H[r-k][k]
                nc.vector.reduce_sum(
                    sub_ap(H, (t0 + HB - 1) * W + 1, [[W, n], [1 - W, 15]]),
                    sub_ap(P, 0, [[60, n], [4, 15], [1, 4]]),
                    axis=X,
                )
                # G16 (sensors 63..66): H[r-16][16]
                nc.vector.reduce_sum(
                    sub_ap(H, (t0 + HB - 16) * W + 16, [[W, n]]),
                    sub_ap(ring, base + 63, [[S, n], [1, 4]]),
                    axis=X,
                )
                # G0 (sensors 0..2): H[r][0]
                nc.vector.reduce_sum(
                    sub_ap(H, (t0 + HB) * W, [[W, n]]),
                    sub_ap(ring, base, [[S, n], [1, 3]]),
                    axis=X,
                )
                # sensor 127 (d=32): H[r-32][17], on ACT
                nc.scalar.copy(
                    sub_ap(H, (t0 + HB - 32) * W + 17, [[W, n]]),
                    sub_ap(ring, base + 127, [[S, n]]),
                )

            def diag(t0, t1, out_sb):
                n = t1 - t0
                o = out_sb[:, t0:t1]
                nc.vector.reduce_sum(
                    o,
                    sub_ap(H, (t0 + HB) * W, [[W, n], [1, W]]),
                    axis=X,
                )
                nc.scalar.mul(o, o, SCALE)

            for b in range(BC):
                out_sb = o_pool.tile([128, LS], f32, tag="out_sb")

                # chunk 0 first: its skewed writes fill H pre-rows [-32, 0)
                # which feed the partition-shifted tail; its fold tail (rows
                # [40, 56), needing rows [56, 72)) is deferred to batch end.
                slot0 = load(b, 0, LC)
                P0 = p_pool.tile([128, LC * 60], f32, tag="P")
                fold1(slot0, P0, LC)
                # chunk 0's slot is recycled before its fold tail can run
                # (it needs chunk 1, loaded last): save rows [40,56) cols 3:63
                nc.gpsimd.tensor_copy(
                    c0sav[:],
                    sub_ap(ring, slot0 * LC * S + (LC - 16) * S + 3, [[S, 16], [1, 60]]),
                )
                # stage: partition p gets p+1's rows [0, 16) for the last
                # chunk's fold (reaches rows [784, 800))
                nc.scalar.dma_start(
                    stage[0:127, :],
                    ring[1:128, slot0 * LC * S : slot0 * LC * S + 16 * S],
                )
                # chunk-0 G-pass on rows [0, 40) only (P rows [40,56) missing)
                nc.vector.reduce_sum(
                    sub_ap(H, (HB - 1) * W + 1, [[W, LC - 16], [1 - W, 15]]),
                    sub_ap(P0, 0, [[60, LC - 16], [4, 15], [1, 4]]),
                    axis=X,
                )
                nc.vector.reduce_sum(
                    sub_ap(H, (HB - 16) * W + 16, [[W, LC]]),
                    sub_ap(ring, slot0 * LC * S + 63, [[S, LC], [1, 4]]),
                    axis=X,
                )
                nc.vector.reduce_sum(
                    sub_ap(H, HB * W, [[W, LC]]),
                    sub_ap(ring, slot0 * LC * S, [[S, LC], [1, 3]]),
                    axis=X,
                )
                nc.scalar.copy(
                    sub_ap(H, (HB - 32) * W + 17, [[W, LC]]),
                    sub_ap(ring, slot0 * LC * S + 127, [[S, LC]]),
                )
                # H halo: partition p's tail rows [752, 784) = p+1's pre-rows
                nc.scalar.dma_start(
                    H[0:127, LS * W : (LS + HB) * W],
                    H[1:128, 0 : HB * W],
                )

                slot_above = None  # slot holding rows [t1, t1+56)
                for c in range(NCH - 1, 0, -1):
                    t0, t1 = c * LC, (c + 1) * LC
                    slot = load(b, t0, t1)
                    P = p_pool.tile([128, LC * 60], f32, tag="P")
                    fold1(slot, P, LC)
                    if c == NCH - 1:
                        hi = sub_ap(stage, 67, [[S, 16], [1, 60]])
                    else:
                        hi = sub_ap(ring, slot_above * LC * S + 67, [[S, 16], [1, 60]])
                    fold2(slot, LC - 16, P, LC - 16, hi)
                    g_pass(t0, t1, slot, P)
                    diag(t0, t1, out_sb)
                    if t0 == 392:
                        # top half of the batch output is complete
                        nc.scalar.dma_start(
                            bass.AP(
                                y.ap().tensor, b * TP + 392, [[LS, 128], [1, LS - 392]]
                            ),
                            out_sb[:, 392:LS],
                        )
                    slot_above = slot

                # chunk 0's deferred fold tail: rows [40, 56) (saved in c0sav),
                # +16 rows now in chunk 1's slot; then its F_k rows, final diag.
                nc.gpsimd.tensor_add(
                    PB[:],
                    c0sav[:],
                    sub_ap(ring, slot_above * LC * S + 67, [[S, 16], [1, 60]]),
                )
                nc.vector.reduce_sum(
                    sub_ap(H, (HB + LC - 16 - 1) * W + 1, [[W, 16], [1 - W, 15]]),
                    sub_ap(PB, 0, [[60, 16], [4, 15], [1, 4]]),
                    axis=X,
                )
                diag(0, LC, out_sb)
                nc.scalar.dma_start(
                    bass.AP(y.ap().tensor, b * TP, [[LS, 128], [1, 392]]),
                    out_sb[:, 0:392],
                )

    nc.compile()
    return nc


def _get_nc():
    if "nc" not in _cache:
        _cache["nc"] = _build()
    return _cache["nc"]


def kernel(microphone_array: np.ndarray) -> np.ndarray:
    from concourse.bass_utils import run_bass_kernel_spmd

    x = np.asarray(microphone_array, dtype=np.float32)
    assert x.shape == (B, T, S)
    nc = _get_nc()

    in_maps = []
    for c in range(NCORES):
        shard = np.zeros((BC, TP, S), dtype=np.float32)
        shard[:, :T] = x[c * BC : (c + 1) * BC]
        in_maps.append({"x": shard.reshape(-1)})

    res = _cache["res"] = run_bass_kernel_spmd(
        nc, in_maps, core_ids=list(range(NCORES)), trace=_cache.get("trace", False)
    )

    out = np.empty((B, T), dtype=np.float32)
    for c in range(NCORES):
        out[c * BC : (c + 1) * BC] = res.results[c]["y"].reshape(BC, TP)[:, :T]
    return out
